# revision 1
# baseline (speedup 1.0000x reference)
"""Trainium2 Bass kernel for a 2-layer GAT (nn_GAT_83382495084588).

Distribution (8 NeuronCores, pure SPMD — one program, per-core data):
  - dst-node sharding with a free A/B src-designation (greedy-balanced per
    dst) splitting the feature table into two int16-addressable halves;
    nodes lex-sorted by (a, b) counts per designation pool, dealt so every
    core/round tile holds 64 A-rows (partitions 0:63) and 64 B-rows
    (64:127) and all cores share the per-round slot schedule DA[r]/DB[r].
  - Phase 0 (sharded): each core computes z rows only for its own NT nodes
    (h @ [W1 | W1@al_bd | W1@ar_bd], fp16 matmul) writing a compact
    [NT, 768B] shard (feat fp16 512B + el f32 32B + pad); own-node
    el/er/feat stay in SBUF. Two AllGathers per layer exchange the
    A/B-half shards.
  - Edge phases: per round two dma_gathers (wrap16 int16 idx, 768B rows L1
    / 256B rows L2) fetch src feat+el; e = lrelu(el+er) + additive fp16
    mask for padded slots, exp on the scalar engine; self-edges never
    gathered (local feat/el/er + multiplicity m). Layer-2 el is recomputed
    on-chip (feat2 . al2). Messages fp16, accumulated with fp16
    identity-matmuls into PSUM, normalized once by 1/den after
    aggregation.
  - Layer-2 matmul (h1 transpose + W2ext) is fused into the layer-1 round
    loop; shard writes are contiguous (no scatters).

kernel(**inputs) takes the full unsharded inputs and returns the full
(50000, 64) float32 output; host numpy does sharding/index prep + unshuffle.
"""

import os
import sys
from dataclasses import dataclass, field

import numpy as np

for _p in ("/opt/trn_rl_repo", "/root/.axon_site/_ro/trn_rl_repo"):
    if os.path.isdir(_p) and _p not in sys.path:
        sys.path.append(_p)

import concourse.bacc as bacc
import concourse.bass as bass
import concourse.mybir as mybir
import concourse.tile as tile
from concourse.bass import IndirectOffsetOnAxis
from concourse.bass_utils import run_bass_kernel_spmd

F32 = mybir.dt.float32
F16 = mybir.dt.float16
I32 = mybir.dt.int32
I16 = mybir.dt.int16
AF = mybir.ActivationFunctionType
OP = mybir.AluOpType

P = 128
NCC = 8
PHASES = int(os.environ.get("GAT_PHASES", "4"))
GK = int(os.environ.get("GAT_GK", "1"))     # indirect-gather column batch
NEG_SLOPE = 0.2
F16_INPUTS = {"htiles", "W1ext", "W2ext", "alrep", "al2rep", "ident16",
              "maskt"}


@dataclass
class Prob:
    N: int
    IN_DIM: int
    H1: int
    HID: int
    OUT_DIM: int
    rounds: int = 0
    DD: list = field(default_factory=list)
    DA: list = field(default_factory=list)
    DB: list = field(default_factory=list)
    C1: int = 0
    NT: int = 0
    SD: int = 0

    def finish(self):
        self.C1 = self.H1 * self.HID
        self.NT = self.rounds * P
        self.SD = int(sum(self.DD))
        return self


def prep_all(inputs, pr: Prob):
    src = np.asarray(inputs["src"]).astype(np.int64)
    dst = np.asarray(inputs["dst"]).astype(np.int64)
    h = np.asarray(inputs["h"], dtype=np.float32)
    W1 = np.asarray(inputs["W1"], dtype=np.float32)
    al1 = np.asarray(inputs["al1"], dtype=np.float32)
    ar1 = np.asarray(inputs["ar1"], dtype=np.float32)
    b1 = np.asarray(inputs["b1"], dtype=np.float32)
    W2 = np.asarray(inputs["W2"], dtype=np.float32)
    al2 = np.asarray(inputs["al2"], dtype=np.float32)
    ar2 = np.asarray(inputs["ar2"], dtype=np.float32)
    b2 = np.asarray(inputs["b2"], dtype=np.float32)
    N = pr.N

    selfmask = src == dst
    m_cnt = np.bincount(dst[selfmask], minlength=N)      # self multiplicity
    ns_src = src[~selfmask]
    ns_dst = dst[~selfmask]
    deg = np.bincount(ns_dst, minlength=N)               # non-self in-degree

    NT_G = ((N + NCC * P - 1) // (NCC * P)) * (NCC * P)
    rounds = NT_G // (NCC * P)
    n_dummy = NT_G - N
    HALFT = NT_G // 2        # rows per table half
    halfNT = NT_G // (2 * NCC)   # rows per core per half

    # greedy A/B designation balancing each dst's in-neighbor split
    o2 = np.argsort(ns_src, kind="stable")
    odst = ns_dst[o2]
    ost = np.zeros(N + 1, np.int64)
    np.cumsum(np.bincount(ns_src, minlength=N), out=ost[1:])
    imb = np.zeros(N, np.int64)
    desA = np.zeros(N, bool)
    cntA = 0
    for v in np.random.default_rng(2).permutation(N):
        nbrs = odst[ost[v]:ost[v + 1]]
        s = np.sign(imb[nbrs]).sum()
        if (s < 0 or (s == 0 and cntA < N // 2)) and cntA < N // 2:
            desA[v] = True
            cntA += 1
            imb[nbrs] += 1
        else:
            imb[nbrs] -= 1
    # refinement sweeps: flip designation where it reduces total |a-b|
    # (pool sizes may drift within the dummy slack of +-88)
    lo_cnt, hi_cnt = N // 2 - 80, N // 2 + 80
    for sweep in range(2):
        for v in np.random.default_rng(3 + sweep).permutation(N):
            nbrs = odst[ost[v]:ost[v + 1]]
            if len(nbrs) == 0:
                continue
            base = imb[nbrs] - (1 if desA[v] else -1)
            sA = np.abs(base + 1).sum()
            sB = np.abs(base - 1).sum()
            want_A = sA < sB
            if want_A == desA[v]:
                continue
            if want_A and cntA >= hi_cnt:
                continue
            if not want_A and cntA <= lo_cnt:
                continue
            desA[v] = want_A
            if want_A:
                imb[nbrs] = base + 1
                cntA += 1
            else:
                imb[nbrs] = base - 1
                cntA -= 1
    a_cnt = np.bincount(ns_dst[desA[ns_src]], minlength=N)
    b_cnt = deg - a_cnt

    # pools: A-designated real nodes lex-sorted by (a, b), padded with
    # dummies (-1) to HALFT; B likewise
    selA = np.nonzero(desA)[0]
    selB = np.nonzero(~desA)[0]
    pa = selA[np.lexsort((b_cnt[selA], a_cnt[selA]))]
    pb = selB[np.lexsort((b_cnt[selB], a_cnt[selB]))]
    pa = np.concatenate([pa, np.full(HALFT - len(pa), -1, np.int64)])
    pb = np.concatenate([pb, np.full(HALFT - len(pb), -1, np.int64)])

    HW = NCC * 64   # pool window per round (512)
    DA = np.zeros(rounds, np.int64)
    DB = np.zeros(rounds, np.int64)
    for r in range(rounds):
        w = np.concatenate([pa[r * HW:(r + 1) * HW], pb[r * HW:(r + 1) * HW]])
        w = w[w >= 0]
        DA[r] = max(1, int(a_cnt[w].max()) if len(w) else 1)
        DB[r] = max(1, int(b_cnt[w].max()) if len(w) else 1)

    pr.rounds = rounds
    pr.DD = [int(DA[i] + DB[i]) for i in range(rounds)]
    pr.DA = [int(x) for x in DA]
    pr.DB = [int(x) for x in DB]
    pr.finish()
    offs = np.zeros(rounds + 1, np.int64)
    np.cumsum(np.array(pr.DD), out=offs[1:])

    # core_nodes: partitions 0:64 = A-pool block, 64:128 = B-pool block
    core_nodes = np.zeros((NCC, pr.NT), np.int64)
    for r in range(rounds):
        for c in range(NCC):
            blk = (c + r) % NCC
            core_nodes[c, r * P:r * P + 64] = \
                pa[r * HW + blk * 64:r * HW + (blk + 1) * 64]
            core_nodes[c, r * P + 64:(r + 1) * P] = \
                pb[r * HW + blk * 64:r * HW + (blk + 1) * 64]

    # node -> row within its half (A half and B half each HALFT rows)
    posh = np.full(N, -1, np.int64)
    for c in range(NCC):
        nn = core_nodes[c]
        for r in range(rounds):
            za = nn[r * P:r * P + 64]
            zb = nn[r * P + 64:(r + 1) * P]
            va = za >= 0
            vb = zb >= 0
            posh[za[va]] = c * halfNT + r * 64 + np.nonzero(va)[0]
            posh[zb[vb]] = c * halfNT + r * 64 + np.nonzero(vb)[0]

    # CSR over non-self edges by dst
    sort = np.argsort(ns_dst, kind="stable")
    s_src = ns_src[sort]
    starts = np.zeros(N + 1, np.int64)
    np.cumsum(deg, out=starts[1:])

    def wrap16(flat_idx):
        n = len(flat_idx)
        S = max(1, (n + 15) // 16)
        t = np.zeros((16, S), np.int16)
        ii = np.arange(n)
        t[ii % 16, ii // 16] = flat_idx
        return np.tile(t, (8, 1))

    per_core = []
    for c in range(NCC):
        nodes = core_nodes[c]
        mask = np.full((P, pr.SD), np.float16(-60000.0), np.float16)
        mrow = np.ones((P, rounds), np.float32)
        gi = []
        for r in range(rounds):
            da, db = pr.DA[r], pr.DB[r]
            o = offs[r]
            iA = np.zeros((da, P), np.int64)
            iB = np.zeros((db, P), np.int64)
            for p in range(P):
                n = nodes[r * P + p]
                if n < 0:
                    continue
                mrow[p, r] = max(1, int(m_cnt[n]))
                ss = s_src[starts[n]:starts[n + 1]]
                sa = ss[desA[ss]]
                sb = ss[~desA[ss]]
                iA[0:len(sa), p] = posh[sa]
                iB[0:len(sb), p] = posh[sb]
                mask[p, o:o + len(sa)] = 0.0
                mask[p, o + da:o + da + len(sb)] = 0.0
            gi.append(iA.reshape(-1))
            gi.append(iB.reshape(-1))
        gidx = wrap16(np.concatenate(gi)).astype(np.int16)
        per_core.append(dict(gidx=gidx, maskt=mask, mrow=mrow))

    # h tiles per core: own nodes' h rows, fp16, transposed for matmul lhsT
    # layout [P(k within chunk), rounds*2(chunk-major per round), P(node)]
    h_own = np.zeros((NCC, pr.NT, pr.IN_DIM), np.float32)
    for c in range(NCC):
        nn = core_nodes[c]
        valid = nn >= 0
        h_own[c, valid] = h[nn[valid]]
    ht = h_own.reshape(NCC, rounds, P, 2, P).transpose(0, 4, 1, 3, 2)
    ht = np.ascontiguousarray(ht.reshape(NCC, P, rounds * 2, P)
                              .astype(np.float16))

    # extended weights (f64 host precompute)
    H1n, HID = pr.H1, pr.HID
    al_bd = np.zeros((pr.C1, H1n), np.float64)
    ar_bd = np.zeros((pr.C1, H1n), np.float64)
    for hh in range(H1n):
        al_bd[hh * HID:(hh + 1) * HID, hh] = al1[hh].astype(np.float64)
        ar_bd[hh * HID:(hh + 1) * HID, hh] = ar1[hh].astype(np.float64)
    W1f = W1.astype(np.float64)
    W1ext = np.concatenate([W1, (W1f @ al_bd).astype(np.float32),
                            (W1f @ ar_bd).astype(np.float32)], axis=1)
    W2f = W2.astype(np.float64)
    W2ext = np.concatenate(
        [W2, (W2f @ al2.astype(np.float64).reshape(-1, 1)).astype(np.float32),
         (W2f @ ar2.astype(np.float64).reshape(-1, 1)).astype(np.float32)],
        axis=1)

    shared = dict(
        W1ext=np.ascontiguousarray(W1ext.astype(np.float16)),
        W2ext=np.ascontiguousarray(W2ext.astype(np.float16)),
        alrep=np.broadcast_to(al1.reshape(1, pr.C1).astype(np.float16),
                              (P, pr.C1)).copy(),
        al2rep=np.broadcast_to(al2.reshape(1, pr.OUT_DIM).astype(np.float16),
                               (P, pr.OUT_DIM)).copy(),
        ident16=np.eye(P, dtype=np.float16),
        b1rep=np.broadcast_to(b1, (P, pr.C1)).copy(),
        b2rep=np.broadcast_to(b2, (P, pr.OUT_DIM)).copy(),
    )
    in_maps = []
    for c in range(NCC):
        mm = dict(shared)
        mm.update(per_core[c])
        mm["htiles"] = ht[c]
        in_maps.append(mm)
    sched = dict(core_nodes=core_nodes, rounds=rounds)
    return sched, in_maps


def build_kernel_fn(pr: Prob):
    rounds, DD = pr.rounds, pr.DD
    DA, DB = pr.DA, pr.DB
    C1, H1, HID, OUT = pr.C1, pr.H1, pr.HID, pr.OUT_DIM
    NT = pr.NT
    halfNT = NT // 2
    HALFT = NCC * halfNT
    Z1W = C1 // 2 + 64     # row: feat fp16 (512B) + el f32 (32B) + pad
    Z2W = OUT             # fp16 feat2 (128B) + pad -> 256B rows for dma_gather
    offs = np.zeros(rounds + 1, np.int64)
    np.cumsum(np.array(DD), out=offs[1:])
    gi_off = [0]
    for r in range(rounds):
        gi_off.append(gi_off[-1] + (DA[r] + DB[r]) * 8)

    dbg = int(os.environ.get("GAT_DEBUG", "0"))

    def kern(tc: tile.TileContext, outs, ins):
        nc = tc.nc

        z1shard = nc.dram_tensor("z1shardd", [NT, Z1W], F32)
        featod = nc.dram_tensor("featod", [rounds, P, C1 // 2], F32)
        z2shard = nc.dram_tensor("z2shardd", [NT, Z2W], F32)
        Z1 = nc.dram_tensor("Z1d", [NCC * NT, Z1W], F32, addr_space="Shared")
        Z2 = nc.dram_tensor("Z2d", [NCC * NT, Z2W], F32, addr_space="Shared")
        if dbg:
            d1 = nc.dram_tensor("dbg1", [NCC * NT, Z1W], F32,
                                kind="ExternalOutput")
            d2 = nc.dram_tensor("dbg2", [NCC * NT, Z2W], F32,
                                kind="ExternalOutput")
        if dbg >= 2:
            dg = nc.dram_tensor("dbgg", [P, DD[0], Z1W], F32,
                                kind="ExternalOutput")
            dew = nc.dram_tensor("dbgew", [P, DD[0] + 1, H1], F32,
                                 kind="ExternalOutput")
            dh1 = nc.dram_tensor("dbgh1", [P, rounds, C1], F32,
                                 kind="ExternalOutput")

        with (
            tc.tile_pool(name="const", bufs=1) as cpool,
            tc.tile_pool(name="big", bufs=1) as big,
        ):
            # ---- constants ----
            w1e = cpool.tile([P, 2, C1 + 2 * H1], F16)
            for c in range(2):
                nc.sync.dma_start(w1e[:, c, :], ins["W1ext"][c * P:(c + 1) * P, :])
            w2e = cpool.tile([P, 2, OUT + 2], F16)
            for c in range(2):
                nc.sync.dma_start(w2e[:, c, :], ins["W2ext"][c * P:(c + 1) * P, :])
            ident16 = cpool.tile([P, P], F16)
            nc.sync.dma_start(ident16[:], ins["ident16"][:, :])
            alrep = cpool.tile([P, C1], F16)
            nc.sync.dma_start(alrep[:], ins["alrep"][:, :])
            al2rep = cpool.tile([P, OUT], F16)
            nc.sync.dma_start(al2rep[:], ins["al2rep"][:, :])
            b1r = cpool.tile([P, C1], F32)
            nc.sync.dma_start(b1r[:], ins["b1rep"][:, :])
            b2r = cpool.tile([P, OUT], F32)
            nc.sync.dma_start(b2r[:], ins["b2rep"][:, :])
            gidx = cpool.tile([P, gi_off[-1]], I16)
            nc.sync.dma_start(gidx[:], ins["gidx"][:, :])
            maskt = cpool.tile([P, pr.SD], F16)
            nc.sync.dma_start(maskt[:], ins["maskt"][:, :])
            mrow = cpool.tile([P, rounds], F32)
            nc.sync.dma_start(mrow[:], ins["mrow"][:, :])

            el_own = big.tile([P, rounds, H1], F32)
            er_own = big.tile([P, rounds, H1], F32)
            feat2_own = big.tile([P, rounds, OUT], F16)
            eler2_own = big.tile([P, rounds, 2], F32)

            if PHASES < 1:
                return
            # ---- phase 0: z1 shard = h_own @ [W1|W1al|W1ar] (fp16) ----
            with (
                nc.named_scope("p0"),
                tc.tile_pool(name="p0h", bufs=4) as p0h,
                tc.tile_pool(name="p0ps", bufs=4, space="PSUM") as p0ps,
                tc.tile_pool(name="p0z", bufs=4) as p0z,
            ):
                for r in range(rounds):
                    htl = p0h.tile([P, 2, P], F16, tag="ht")
                    nc.sync.dma_start(htl[:], ins["htiles"][:, 2 * r:2 * r + 2, :])
                    zps = p0ps.tile([P, C1 + 2 * H1], F32)
                    for c in range(2):
                        nc.tensor.matmul(zps[:], lhsT=htl[:, c, :],
                                         rhs=w1e[:, c, :], start=(c == 0),
                                         stop=(c == 1))
                    zsb = p0z.tile([P, Z1W], F32, tag="zsb")
                    nc.vector.tensor_copy(zsb[:, 0:C1 // 2].bitcast(F16),
                                          zps[:, 0:C1])
                    nc.vector.tensor_copy(zsb[:, C1 // 2:C1 // 2 + H1],
                                          zps[:, C1:C1 + H1])
                    nc.vector.tensor_copy(el_own[:, r, :], zps[:, C1:C1 + H1])
                    nc.vector.tensor_copy(er_own[:, r, :],
                                          zps[:, C1 + H1:C1 + 2 * H1])
                    nc.sync.dma_start(featod[r, :, :], zsb[:, 0:C1 // 2])
                    nc.sync.dma_start(z1shard[r * 64:(r + 1) * 64, :],
                                      zsb[0:64, :])
                    nc.sync.dma_start(
                        z1shard[halfNT + r * 64:halfNT + (r + 1) * 64, :],
                        zsb[64:P, :])

            with nc.named_scope("ag1"):
                nc.gpsimd.collective_compute(
                    "AllGather", OP.bypass, replica_groups=[list(range(NCC))],
                    ins=[z1shard[0:halfNT, :]], outs=[Z1[0:HALFT, :]])
                nc.gpsimd.collective_compute(
                    "AllGather", OP.bypass, replica_groups=[list(range(NCC))],
                    ins=[z1shard[halfNT:NT, :]], outs=[Z1[HALFT:2 * HALFT, :]])

            if PHASES < 2:
                return
            # ---- layer-1 edge phase (+ fused layer-2 matmul) ----
            with (
                nc.named_scope("l1edge"),
                tc.tile_pool(name="fg", bufs=4) as fgp,
                tc.tile_pool(name="ew", bufs=4) as ewp,
                tc.tile_pool(name="msg", bufs=2) as msgp,
                tc.tile_pool(name="l1ps", bufs=3, space="PSUM") as l1ps,
                tc.tile_pool(name="ep", bufs=4) as epp,
                tc.tile_pool(name="tps", bufs=3, space="PSUM") as tpsp,
                tc.tile_pool(name="h1t", bufs=3) as h1tp,
                tc.tile_pool(name="z2ps", bufs=2, space="PSUM") as z2psp,
                tc.tile_pool(name="z2s", bufs=4) as z2sp,
            ):
                for r in range(rounds):
                    dd = DD[r]
                    da, db = DA[r], DB[r]
                    o = int(offs[r])
                    c0 = gi_off[r]
                    fo = ewp.tile([P, C1 // 2], F32, tag="fo")
                    nc.sync.dma_start(fo[:], featod[r, :, :])
                    g = fgp.tile([P, dd, Z1W], F32, tag="g")
                    nc.gpsimd.dma_gather(
                        g[:, 0:da, :], Z1[0:HALFT, :],
                        gidx[:, c0:c0 + da * 8], da * P, da * P, Z1W,
                        single_packet=False)
                    nc.gpsimd.dma_gather(
                        g[:, da:dd, :], Z1[HALFT:2 * HALFT, :],
                        gidx[:, c0 + da * 8:c0 + dd * 8], db * P, db * P, Z1W,
                        single_packet=False)
                    # el rides in the gathered rows (f32 words after feat)
                    mg = msgp.tile([P, dd + 1, C1], F16, tag="mg")
                    ew = ewp.tile([P, dd + 1, H1], F32, tag="ew")
                    nc.vector.tensor_copy(ew[:, 0:dd, :],
                                          g[:, :, C1 // 2:C1 // 2 + H1])
                    nc.vector.tensor_copy(ew[:, dd, :], el_own[:, r, :])
                    nc.vector.tensor_tensor(
                        out=ew[:], in0=ew[:],
                        in1=er_own[:, r, None, :].to_broadcast((P, dd + 1, H1)),
                        op=OP.add)
                    nc.vector.tensor_tensor(
                        out=ew[:, 0:dd, :], in0=ew[:, 0:dd, :],
                        in1=maskt[:, o:o + dd, None].to_broadcast((P, dd, H1)),
                        op=OP.add)
                    lr = ewp.tile([P, dd + 1, H1], F32, tag="lr")
                    nc.vector.tensor_scalar_mul(lr[:], ew[:], NEG_SLOPE)
                    nc.vector.tensor_tensor(out=ew[:], in0=ew[:], in1=lr[:],
                                            op=OP.max)
                    nc.scalar.activation(out=ew[:], in_=ew[:], func=AF.Exp)
                    nc.vector.tensor_tensor(
                        out=ew[:, dd, :], in0=ew[:, dd, :],
                        in1=mrow[:, r:r + 1].to_broadcast((P, H1)), op=OP.mult)
                    den = ewp.tile([P, H1], F32, tag="den")
                    nc.vector.reduce_sum(
                        out=den[:], in_=ew[:].rearrange("p d h -> p h d"),
                        axis=mybir.AxisListType.X)
                    nc.vector.reciprocal(out=den[:], in_=den[:])
                    # messages (fp16) + identity-matmul accumulation
                    nc.vector.tensor_tensor(
                        out=mg[:, 0:dd, :].rearrange("p d (h w) -> p d h w",
                                                     h=H1),
                        in0=g[:, :, 0:C1 // 2].bitcast(F16).rearrange(
                            "p d (h w) -> p d h w", h=H1),
                        in1=ew[:, 0:dd, :, None].to_broadcast((P, dd, H1, HID)),
                        op=OP.mult)
                    nc.vector.tensor_tensor(
                        out=mg[:, dd, :].rearrange("p (h w) -> p h w", h=H1),
                        in0=fo[:].bitcast(F16).rearrange("p (h w) -> p h w",
                                                          h=H1),
                        in1=ew[:, dd, :, None].to_broadcast((P, H1, HID)),
                        op=OP.mult)
                    ps = l1ps.tile([P, C1], F32)
                    for k in range(dd + 1):
                        nc.tensor.matmul(ps[:], lhsT=ident16[:], rhs=mg[:, k, :],
                                         start=(k == 0), stop=(k == dd))
                    # h1 = elu(ps/den + b1)
                    x = epp.tile([P, C1], F32, tag="x")
                    nc.vector.tensor_tensor(
                        out=x[:].rearrange("p (h w) -> p h w", h=H1),
                        in0=ps[:].rearrange("p (h w) -> p h w", h=H1),
                        in1=den[:, :, None].to_broadcast((P, H1, HID)),
                        op=OP.mult)
                    nc.vector.tensor_tensor(out=x[:], in0=x[:], in1=b1r[:],
                                            op=OP.add)
                    mn = epp.tile([P, C1], F32, tag="mn")
                    nc.vector.tensor_scalar_min(mn[:], x[:], 0.0)
                    nc.scalar.activation(out=mn[:], in_=mn[:], func=AF.Exp)
                    nc.vector.tensor_scalar_max(x[:], x[:], 0.0)
                    nc.vector.tensor_tensor(out=x[:], in0=x[:], in1=mn[:],
                                            op=OP.add)
                    h1r = epp.tile([P, C1], F16, tag="h1r")
                    nc.vector.tensor_scalar_sub(h1r[:], x[:], 1.0)
                    if dbg >= 2:
                        nc.sync.dma_start(dh1[:, r, :], x[:])
                        if r == 0:
                            nc.sync.dma_start(dg[:, :, :], g[:])
                            nc.sync.dma_start(dew[:, :, :], ew[:])
                    # fused layer-2 matmul for this round
                    tps = tpsp.tile([P, 2, P], F16)
                    for c in range(2):
                        nc.tensor.transpose(out=tps[:, c, :],
                                            in_=h1r[:, c * P:(c + 1) * P],
                                            identity=ident16[:])
                    h1t = h1tp.tile([P, 2, P], F16, tag="h1t")
                    nc.vector.tensor_copy(h1t[:], tps[:])
                    z2ps = z2psp.tile([P, OUT + 2], F32)
                    for c in range(2):
                        nc.tensor.matmul(z2ps[:], lhsT=h1t[:, c, :],
                                         rhs=w2e[:, c, :],
                                         start=(c == 0), stop=(c == 1))
                    zsb2 = z2sp.tile([P, Z2W], F32, tag="zsb2")
                    nc.vector.tensor_copy(zsb2[:, 0:OUT // 2].bitcast(F16),
                                          z2ps[:, 0:OUT])
                    nc.vector.tensor_copy(eler2_own[:, r, :],
                                          z2ps[:, OUT:OUT + 2])
                    nc.vector.tensor_copy(feat2_own[:, r, :],
                                          zsb2[:, 0:OUT // 2].bitcast(F16))
                    nc.sync.dma_start(z2shard[r * 64:(r + 1) * 64, :],
                                      zsb2[0:64, :])
                    nc.sync.dma_start(
                        z2shard[halfNT + r * 64:halfNT + (r + 1) * 64, :],
                        zsb2[64:P, :])

            with nc.named_scope("ag2"):
                nc.gpsimd.collective_compute(
                    "AllGather", OP.bypass, replica_groups=[list(range(NCC))],
                    ins=[z2shard[0:halfNT, :]], outs=[Z2[0:HALFT, :]])
                nc.gpsimd.collective_compute(
                    "AllGather", OP.bypass, replica_groups=[list(range(NCC))],
                    ins=[z2shard[halfNT:NT, :]], outs=[Z2[HALFT:2 * HALFT, :]])
            if dbg:
                nc.sync.dma_start(d1[:, :], Z1[:, :])
                nc.sync.dma_start(d2[:, :], Z2[:, :])

            if PHASES < 3:
                return
            # ---- layer-2 edge phase ----
            with (
                nc.named_scope("l2edge"),
                tc.tile_pool(name="fg2", bufs=8) as fg2p,
                tc.tile_pool(name="ew2", bufs=6) as ew2p,
                tc.tile_pool(name="msg2", bufs=6) as msg2p,
                tc.tile_pool(name="l2ps", bufs=4, space="PSUM") as l2ps,
            ):
                for r in range(rounds):
                    dd = DD[r]
                    da, db = DA[r], DB[r]
                    o = int(offs[r])
                    c0 = gi_off[r]
                    g2 = fg2p.tile([P, dd, Z2W], F32, tag="g2")
                    nc.gpsimd.dma_gather(
                        g2[:, 0:da, :], Z2[0:HALFT, :],
                        gidx[:, c0:c0 + da * 8], da * P, da * P, Z2W,
                        single_packet=False)
                    nc.gpsimd.dma_gather(
                        g2[:, da:dd, :], Z2[HALFT:2 * HALFT, :],
                        gidx[:, c0 + da * 8:c0 + dd * 8], db * P, db * P, Z2W,
                        single_packet=False)
                    mg = msg2p.tile([P, dd + 1, OUT], F16, tag="mg2")
                    nc.vector.tensor_tensor(
                        out=mg[:, 0:dd, :],
                        in0=g2[:, :, 0:OUT // 2].bitcast(F16),
                        in1=al2rep[:, None, :].to_broadcast((P, dd, OUT)),
                        op=OP.mult)
                    ew = ew2p.tile([P, dd + 1], F32, tag="ew2")
                    nc.vector.reduce_sum(out=ew[:, 0:dd], in_=mg[:, 0:dd, :],
                                         axis=mybir.AxisListType.X)
                    nc.vector.tensor_copy(ew[:, dd:dd + 1],
                                          eler2_own[:, r, 0:1])
                    nc.vector.tensor_tensor(
                        out=ew[:], in0=ew[:],
                        in1=eler2_own[:, r, 1:2].to_broadcast((P, dd + 1)),
                        op=OP.add)
                    nc.vector.tensor_tensor(
                        out=ew[:, 0:dd], in0=ew[:, 0:dd],
                        in1=maskt[:, o:o + dd], op=OP.add)
                    lr2 = ew2p.tile([P, dd + 1], F32, tag="lr2")
                    nc.vector.tensor_scalar_mul(lr2[:], ew[:], NEG_SLOPE)
                    nc.vector.tensor_tensor(out=ew[:], in0=ew[:], in1=lr2[:],
                                            op=OP.max)
                    nc.scalar.activation(out=ew[:], in_=ew[:], func=AF.Exp)
                    nc.vector.tensor_tensor(
                        out=ew[:, dd:dd + 1], in0=ew[:, dd:dd + 1],
                        in1=mrow[:, r:r + 1], op=OP.mult)
                    den = ew2p.tile([P, 1], F32, tag="den2")
                    nc.vector.reduce_sum(out=den[:], in_=ew[:],
                                         axis=mybir.AxisListType.X)
                    nc.vector.reciprocal(out=den[:], in_=den[:])
                    nc.vector.tensor_tensor(
                        out=mg[:, 0:dd, :],
                        in0=g2[:, :, 0:OUT // 2].bitcast(F16),
                        in1=ew[:, 0:dd, None].to_broadcast((P, dd, OUT)),
                        op=OP.mult)
                    nc.vector.tensor_tensor(
                        out=mg[:, dd, :], in0=feat2_own[:, r, :],
                        in1=ew[:, dd:dd + 1].to_broadcast((P, OUT)),
                        op=OP.mult)
                    ps = l2ps.tile([P, OUT], F32)
                    for k in range(dd + 1):
                        nc.tensor.matmul(ps[:], lhsT=ident16[:], rhs=mg[:, k, :],
                                         start=(k == 0), stop=(k == dd))
                    ot = ew2p.tile([P, OUT], F32, tag="ot")
                    nc.vector.tensor_tensor(
                        out=ot[:], in0=ps[:],
                        in1=den[:].to_broadcast((P, OUT)), op=OP.mult)
                    nc.vector.tensor_tensor(out=ot[:], in0=ot[:], in1=b2r[:],
                                            op=OP.add)
                    nc.sync.dma_start(outs["out"][:, r, :], ot[:])

    return kern


def declare_io(nc, in_maps, pr: Prob):
    ins_ap = {}
    for k, v in in_maps[0].items():
        if k in F16_INPUTS:
            dt = F16
        else:
            dt = mybir.dt.from_np(v.dtype)
        ins_ap[k] = nc.dram_tensor(
            f"in_{k}", list(v.shape), dt, kind="ExternalInput").ap()
    outs_ap = {"out": nc.dram_tensor(
        "out", [P, pr.rounds, pr.OUT_DIM], F32, kind="ExternalOutput").ap()}
    return ins_ap, outs_ap


def assemble_output(results, sched, pr: Prob):
    out = np.zeros((pr.N, pr.OUT_DIM), np.float32)
    for c in range(NCC):
        nodes = sched["core_nodes"][c]
        oc = results[c]["out"]  # [P, rounds, OUT]
        oc = oc.transpose(1, 0, 2).reshape(pr.NT, pr.OUT_DIM)
        valid = nodes >= 0
        out[nodes[valid]] = oc[valid]
    return out


def _build_and_run(inputs, trace=False):
    pr = Prob(N=50000, IN_DIM=256, H1=8, HID=32, OUT_DIM=64)
    sched, in_maps = prep_all(inputs, pr)
    nc = bacc.Bacc("TRN2", target_bir_lowering=False, debug=False,
                   num_devices=NCC)
    ins_ap, outs_ap = declare_io(nc, in_maps, pr)
    kern = build_kernel_fn(pr)
    with tile.TileContext(nc) as tc:
        kern(tc, outs_ap, ins_ap)
    nc.compile()
    maps = [{f"in_{k}": v for k, v in m.items()} for m in in_maps]
    res = run_bass_kernel_spmd(nc, maps, core_ids=list(range(NCC)),
                               trace=trace)
    return res, sched, pr


def kernel(**inputs) -> np.ndarray:
    res, sched, pr = _build_and_run(inputs, trace=False)
    return assemble_output(res.results, sched, pr)


def kernel_timed(inputs):
    import time
    trace = bool(int(os.environ.get("GAT_TRACE", "0")))
    if trace:
        import trace_hook
        trace_hook.install()
    t0 = time.perf_counter()
    res, sched, pr = _build_and_run(inputs, trace=trace)
    t1 = time.perf_counter()
    print(f"build+run {t1-t0:.1f}s")
    if trace:
        print(f"exec_time_ns: {res.exec_time_ns}")
        if res.per_core_scope_times:
            for scope, d in sorted(res.per_core_scope_times.items()):
                print(f"  scope {scope}: "
                      + " ".join(f"c{c}={v}" for c, v in sorted(d.items())))
        if res.instructions_and_trace:
            print("trace path:", res.instructions_and_trace[1])
    return assemble_output(res.results, sched, pr)


if __name__ == "__main__":
    import pickle
    with open("/tmp/inputs.pkl", "rb") as f:
        inputs = pickle.load(f)
    out = kernel_timed(inputs)
    exp = np.load("/tmp/expected_np.npy")
    rel = np.linalg.norm(out - exp) / np.linalg.norm(exp)
    print("Relative error:", rel)



# revision 7
# speedup vs baseline: 1.8424x; 1.8424x over previous
"""Trainium2 Bass kernel for a 2-layer GAT (nn_GAT_83382495084588).

Distribution (8 NeuronCores, pure SPMD — one program, per-core data):
  - dst-node sharding with a parity A/B src-designation splitting the
    feature table into two int16-addressable halves; nodes lex-sorted by
    (a, b) counts per designation pool, dealt so every core/round tile
    holds 64 A-rows (partitions 0:63) and 64 B-rows (64:127) and all cores
    share the per-round slot schedule DA[r]/DB[r].
  - Phase 0 (sharded): each core computes z rows only for its own NT nodes
    (h @ [W1 | W1@al_bd | W1@ar_bd], fp16 matmul); feat/el/er stay in
    SBUF; the fp16 feat rows (512B) are AllGathered per half.
  - Edge phases: per round two dma_gathers (wrap16 int16 idx) fetch src
    feat; el is recomputed on-chip (feat . al); e = lrelu(el + er), then a
    post-lrelu additive mask (-60000 pad / 0 real / ln(m) self) and exp;
    messages accumulated with strided vector reduce_sum; normalized once
    by 1/den. Self edges never gathered (local feat, multiplicity via
    ln(m) in the mask column).
  - Layer-2 matmul (h1 transpose + W2ext) is fused into the layer-1 round
    loop; shard writes are contiguous (no scatters).

Wall-clock layout: jax/axon init runs on a thread from t=0; input
marshaling runs on a thread concurrent with the bass build + PJRT
compile; outputs are downloaded with a single device-to-host transfer.
"""

import os
import sys
import threading

import numpy as np

for _p in ("/opt/trn_rl_repo", "/root/.axon_site/_ro/trn_rl_repo"):
    if os.path.isdir(_p) and _p not in sys.path:
        sys.path.append(_p)

import concourse.bacc as bacc
import concourse.mybir as mybir
import concourse.tile as tile
from concourse import bass2jax

F32 = mybir.dt.float32
F16 = mybir.dt.float16
I16 = mybir.dt.int16
AF = mybir.ActivationFunctionType
OP = mybir.AluOpType

P = 128
NCC = 8
USE_LRELU = bool(int(os.environ.get("GAT_LRELU", "1")))
USE_ACCUM = bool(int(os.environ.get("GAT_ACCUM", "1")))
N = 50000
IN_DIM, HID, H1, OUT = 256, 32, 8, 64
C1 = H1 * HID
NEG_SLOPE = 0.2
NT_G = ((N + NCC * P - 1) // (NCC * P)) * (NCC * P)   # 50176
ROUNDS = NT_G // (NCC * P)                            # 49
NT = ROUNDS * P                                       # 6272 per core
HALFNT = NT // 2                                      # 3136
HALFT = NCC * HALFNT                                  # 25088
HW = NCC * 64                                         # pool window (512)
Z1W = C1                                              # f16 words: 512B rows
Z2W = 2 * OUT                                         # f16 words: 256B rows


def _init_jax(state):
    try:
        import jax
        state["devices"] = jax.devices()
        bass2jax.install_neuronx_cc_hook()
    except Exception as e:  # surfaced at join
        state["jax_err"] = e


# ---------------------------------------------------------------- schedule
def schedule(src, dst):
    selfm = src == dst
    m_cnt = np.bincount(dst[selfm], minlength=N)
    ns_src = src[~selfm]
    ns_dst = dst[~selfm]
    deg = np.bincount(ns_dst, minlength=N)

    desA = (np.arange(N) & 1) == 0          # parity designation
    edgeA = desA[ns_src]
    a_cnt = np.bincount(ns_dst[edgeA], minlength=N)
    b_cnt = deg - a_cnt

    selA = np.nonzero(desA)[0]
    selB = np.nonzero(~desA)[0]
    pa = selA[np.lexsort((b_cnt[selA], a_cnt[selA]))]
    pb = selB[np.lexsort((b_cnt[selB], a_cnt[selB]))]
    pa = np.concatenate([pa, np.full(HALFT - len(pa), -1, np.int64)])
    pb = np.concatenate([pb, np.full(HALFT - len(pb), -1, np.int64)])
    paw = pa.reshape(ROUNDS, HW)
    pbw = pb.reshape(ROUNDS, HW)

    def wmax(cnt, w):
        return np.where(w >= 0, cnt[np.maximum(w, 0)], 0).max(axis=1)

    DA = np.maximum(1, np.maximum(wmax(a_cnt, paw), wmax(a_cnt, pbw)))
    DB = np.maximum(1, np.maximum(wmax(b_cnt, paw), wmax(b_cnt, pbw)))
    DD = DA + DB

    # core_nodes[c, r, p]: p 0:64 = A pool block, 64:128 = B pool block
    r_ar = np.arange(ROUNDS)
    j64 = np.arange(64)
    cn = np.zeros((NCC, ROUNDS, P), np.int64)
    for c in range(NCC):
        blk = (c + r_ar) % NCC
        idx = blk[:, None] * 64 + j64[None, :]
        cn[c, :, 0:64] = paw[r_ar[:, None], idx]
        cn[c, :, 64:P] = pbw[r_ar[:, None], idx]

    valid = cn >= 0
    ci, ri, pi = np.nonzero(valid)
    nodes_v = cn[valid]
    node2c = np.zeros(N, np.int64)
    node2r = np.zeros(N, np.int64)
    node2p = np.zeros(N, np.int64)
    posh = np.zeros(N, np.int64)
    node2c[nodes_v] = ci
    node2r[nodes_v] = ri
    node2p[nodes_v] = pi
    posh[nodes_v] = ci * HALFNT + ri * 64 + (pi % 64)

    # per-(dst, half) edge ranks
    half = (~edgeA).astype(np.int64)
    key = ns_dst * 2 + half
    order = np.argsort(key, kind="stable")
    ks = key[order]
    gstart = np.zeros(2 * N + 1, np.int64)
    np.cumsum(np.bincount(ks, minlength=2 * N), out=gstart[1:])
    k_rank = np.arange(len(ks)) - gstart[ks]
    e_src = ns_src[order]
    e_dst = ns_dst[order]
    e_half = half[order]
    c_e = node2c[e_dst]
    r_e = node2r[e_dst]
    p_e = node2p[e_dst]

    gi_base = np.zeros(ROUNDS + 1, np.int64)
    np.cumsum(DD * P, out=gi_base[1:])
    gi_len = int(gi_base[-1])
    pos = gi_base[r_e] + np.where(e_half == 1, DA[r_e] * P, 0) + k_rank * P + p_e
    stream = np.zeros(NCC * gi_len, np.int16)
    stream[c_e * gi_len + pos] = posh[e_src].astype(np.int16)
    gidx = np.ascontiguousarray(
        stream.reshape(NCC, gi_len // 16, 16).transpose(0, 2, 1))

    moff = np.zeros(ROUNDS + 1, np.int64)
    np.cumsum(DD + 1, out=moff[1:])
    SD2 = int(moff[-1])
    mask = np.full((NCC, P, SD2), np.float16(-60000.0), np.float16)
    mask[:, :, moff[1:] - 1] = np.float16(0.0)            # self columns
    col = moff[r_e] + np.where(e_half == 1, DA[r_e], 0) + k_rank
    mask[c_e, p_e, col] = np.float16(0.0)
    lnm = np.log(np.maximum(m_cnt[nodes_v], 1)).astype(np.float16)
    mask[ci, pi, moff[ri + 1] - 1] = lnm

    return dict(DA=[int(x) for x in DA], DB=[int(x) for x in DB],
                DD=[int(x) for x in DD], moff=moff, gi_base=gi_base,
                gi_len=gi_len, SD2=SD2, core_nodes=cn, gidx=gidx, mask=mask)


# ----------------------------------------------------------------- marshal
def marshal(inputs, sched, state):
    try:
        h = np.asarray(inputs["h"], dtype=np.float32)
        W1 = np.asarray(inputs["W1"], dtype=np.float32)
        al1 = np.asarray(inputs["al1"], dtype=np.float32)
        ar1 = np.asarray(inputs["ar1"], dtype=np.float32)
        b1 = np.asarray(inputs["b1"], dtype=np.float32)
        W2 = np.asarray(inputs["W2"], dtype=np.float32)
        al2 = np.asarray(inputs["al2"], dtype=np.float32)
        ar2 = np.asarray(inputs["ar2"], dtype=np.float32)
        b2 = np.asarray(inputs["b2"], dtype=np.float32)

        al_bd = np.zeros((C1, H1), np.float64)
        ar_bd = np.zeros((C1, H1), np.float64)
        for hh in range(H1):
            al_bd[hh * HID:(hh + 1) * HID, hh] = al1[hh].astype(np.float64)
            ar_bd[hh * HID:(hh + 1) * HID, hh] = ar1[hh].astype(np.float64)
        W1f = W1.astype(np.float64)
        W1ext = np.concatenate([W1, (W1f @ al_bd).astype(np.float32),
                                (W1f @ ar_bd).astype(np.float32)], axis=1)
        W2f = W2.astype(np.float64)
        W2ext = np.concatenate(
            [W2,
             (W2f @ al2.astype(np.float64).reshape(-1, 1)).astype(np.float32),
             (W2f @ ar2.astype(np.float64).reshape(-1, 1)).astype(np.float32)],
            axis=1)

        cn2 = sched["core_nodes"].reshape(-1)
        vv = cn2 >= 0
        h_own = np.zeros((NCC * NT, IN_DIM), np.float32)
        h_own[vv] = h[cn2[vv]]
        ht = (h_own.reshape(NCC, ROUNDS, P, 2, P).transpose(0, 4, 1, 3, 2)
              .reshape(NCC * P, ROUNDS * 2, P).astype(np.float16))

        def rep(x):
            return np.tile(np.asarray(x), (NCC,) + (1,) * (x.ndim - 1))

        state["concat"] = {
            "in_W1ext": rep(W1ext.astype(np.float16)),
            "in_W2ext": rep(W2ext.astype(np.float16)),
            "in_alrep": rep(np.broadcast_to(
                al1.reshape(1, C1).astype(np.float16), (P, C1))),
            "in_al2rep": rep(np.broadcast_to(
                al2.reshape(1, OUT).astype(np.float16), (P, OUT))),
            "in_ident16": rep(np.eye(P, dtype=np.float16)),
            "in_b1rep": rep(np.broadcast_to(b1, (P, C1))),
            "in_b2rep": rep(np.broadcast_to(b2, (P, OUT))),
            "in_gidx": np.ascontiguousarray(
                sched["gidx"].reshape(NCC * 16, -1)),
            "in_maskt": np.ascontiguousarray(
                sched["mask"].reshape(NCC * P, -1)),
            "in_htiles": np.ascontiguousarray(ht),
        }
    except Exception as e:
        state["marshal_err"] = e


# ------------------------------------------------------------------- build
def build_kernel_fn(sched):
    DA, DB, DD = sched["DA"], sched["DB"], sched["DD"]
    moff, gi_base = sched["moff"], sched["gi_base"]
    SD2, gi_len = sched["SD2"], sched["gi_len"]
    S16 = gi_len // 16

    def kern(tc: tile.TileContext, outs, ins):
        nc = tc.nc
        z1shard = nc.dram_tensor("z1shardd", [NT, Z1W], F16)
        z2shard = nc.dram_tensor("z2shardd", [NT, Z2W], F16)
        Z1 = nc.dram_tensor("Z1d", [NCC * NT, Z1W], F16, addr_space="Shared")
        Z2 = nc.dram_tensor("Z2d", [NCC * NT, Z2W], F16, addr_space="Shared")

        with (
            tc.tile_pool(name="const", bufs=1) as cpool,
            tc.tile_pool(name="big", bufs=1) as big,
        ):
            w1e = cpool.tile([P, 2, C1 + 2 * H1], F16)
            for c in range(2):
                nc.sync.dma_start(w1e[:, c, :], ins["W1ext"][c * P:(c + 1) * P, :])
            w2e = cpool.tile([P, 2, OUT + 2], F16)
            for c in range(2):
                nc.sync.dma_start(w2e[:, c, :], ins["W2ext"][c * P:(c + 1) * P, :])
            ident16 = cpool.tile([P, P], F16)
            nc.sync.dma_start(ident16[:], ins["ident16"][:, :])
            alrep = cpool.tile([P, C1], F16)
            nc.sync.dma_start(alrep[:], ins["alrep"][:, :])
            al2rep = cpool.tile([P, OUT], F16)
            nc.sync.dma_start(al2rep[:], ins["al2rep"][:, :])
            b1r = cpool.tile([P, C1], F32)
            nc.sync.dma_start(b1r[:], ins["b1rep"][:, :])
            b2r = cpool.tile([P, OUT], F32)
            nc.sync.dma_start(b2r[:], ins["b2rep"][:, :])
            gidx = cpool.tile([P, S16], I16)
            for c in range(NCC):
                nc.sync.dma_start(gidx[c * 16:(c + 1) * 16, :], ins["gidx"][:, :])
            maskt = cpool.tile([P, SD2], F16)
            nc.sync.dma_start(maskt[:], ins["maskt"][:, :])

            feat_own = big.tile([P, ROUNDS, C1], F16)
            eler_own = big.tile([P, ROUNDS, 2 * H1], F32)
            eself = big.tile([P, ROUNDS, H1], F32)
            feat2_own = big.tile([P, ROUNDS, OUT], F16)
            eler2_own = big.tile([P, ROUNDS, 2], F32)

            # ---- phase 0: feat/el/er = h_own @ [W1|W1al|W1ar] (fp16) ----
            with (
                nc.named_scope("p0"),
                tc.tile_pool(name="p0h", bufs=4) as p0h,
                tc.tile_pool(name="p0ps", bufs=4, space="PSUM") as p0ps,
            ):
                for r in range(ROUNDS):
                    htl = p0h.tile([P, 2, P], F16, tag="ht")
                    nc.sync.dma_start(htl[:], ins["htiles"][:, 2 * r:2 * r + 2, :])
                    zps = p0ps.tile([P, C1 + 2 * H1], F32)
                    for c in range(2):
                        nc.tensor.matmul(zps[:], lhsT=htl[:, c, :],
                                         rhs=w1e[:, c, :], start=(c == 0),
                                         stop=(c == 1))
                    nc.vector.tensor_copy(feat_own[:, r, :], zps[:, 0:C1])
                    nc.vector.tensor_copy(eler_own[:, r, :],
                                          zps[:, C1:C1 + 2 * H1])
                    nc.sync.dma_start(z1shard[r * 64:(r + 1) * 64, :],
                                      feat_own[0:64, r, :])
                    nc.sync.dma_start(
                        z1shard[HALFNT + r * 64:HALFNT + (r + 1) * 64, :],
                        feat_own[64:P, r, :])
            nc.vector.tensor_tensor(out=eself[:], in0=eler_own[:, :, 0:H1],
                                    in1=eler_own[:, :, H1:2 * H1], op=OP.add)

            with nc.named_scope("ag1"):
                nc.gpsimd.collective_compute(
                    "AllGather", OP.bypass, replica_groups=[list(range(NCC))],
                    ins=[z1shard[0:HALFNT, :]], outs=[Z1[0:HALFT, :]])
                nc.gpsimd.collective_compute(
                    "AllGather", OP.bypass, replica_groups=[list(range(NCC))],
                    ins=[z1shard[HALFNT:NT, :]], outs=[Z1[HALFT:2 * HALFT, :]])

            # ---- layer-1 edge phase (+ fused layer-2 matmul) ----
            with (
                nc.named_scope("l1edge"),
                tc.tile_pool(name="fg", bufs=3) as fgp,
                tc.tile_pool(name="tmp", bufs=2) as tmpp,
                tc.tile_pool(name="ew", bufs=4) as ewp,
                tc.tile_pool(name="ep", bufs=4) as epp,
                tc.tile_pool(name="tps", bufs=3, space="PSUM") as tpsp,
                tc.tile_pool(name="h1t", bufs=3) as h1tp,
                tc.tile_pool(name="z2ps", bufs=2, space="PSUM") as z2psp,
            ):
                for r in range(ROUNDS):
                    dd, da, db = DD[r], DA[r], DB[r]
                    o = int(moff[r])
                    c0 = int(gi_base[r]) // 16
                    g = fgp.tile([P, dd + 1, C1], F16, tag="g")
                    nc.gpsimd.dma_gather(
                        g[:, 0:da, :], Z1[0:HALFT, :],
                        gidx[:, c0:c0 + da * 8], da * P, da * P, Z1W,
                        single_packet=False)
                    nc.gpsimd.dma_gather(
                        g[:, da:dd, :], Z1[HALFT:2 * HALFT, :],
                        gidx[:, c0 + da * 8:c0 + dd * 8], db * P, db * P, Z1W,
                        single_packet=False)
                    # el for gathered slots = feat . al (per head)
                    tmp = tmpp.tile([P, dd, C1], F16, tag="tmp")
                    nc.vector.tensor_tensor(
                        out=tmp[:], in0=g[:, 0:dd, :],
                        in1=alrep[:, None, :].to_broadcast((P, dd, C1)),
                        op=OP.mult)
                    ew = ewp.tile([P, dd + 1, H1], F32, tag="ew")
                    nc.vector.reduce_sum(
                        out=ew[:, 0:dd, :],
                        in_=tmp[:].rearrange("p d (h w) -> p d h w", h=H1),
                        axis=mybir.AxisListType.X)
                    nc.vector.tensor_tensor(
                        out=ew[:, 0:dd, :], in0=ew[:, 0:dd, :],
                        in1=eler_own[:, r, None, H1:2 * H1].to_broadcast(
                            (P, dd, H1)),
                        op=OP.add)
                    nc.vector.tensor_copy(ew[:, dd, :], eself[:, r, :])
                    if USE_LRELU:
                        nc.scalar.activation(out=ew[:], in_=ew[:],
                                             func=AF.Prelu, alpha=NEG_SLOPE)
                    else:
                        lr = ewp.tile([P, dd + 1, H1], F32, tag="lr")
                        nc.vector.tensor_scalar_mul(lr[:], ew[:], NEG_SLOPE)
                        nc.vector.tensor_tensor(out=ew[:], in0=ew[:],
                                                in1=lr[:], op=OP.max)
                    nc.vector.tensor_tensor(
                        out=ew[:], in0=ew[:],
                        in1=maskt[:, o:o + dd + 1, None].to_broadcast(
                            (P, dd + 1, H1)),
                        op=OP.add)
                    nc.scalar.activation(out=ew[:], in_=ew[:], func=AF.Exp)
                    den = ewp.tile([P, H1], F32, tag="den")
                    nc.vector.reduce_sum(
                        out=den[:], in_=ew[:].rearrange("p d h -> p h d"),
                        axis=mybir.AxisListType.X)
                    nc.vector.reciprocal(out=den[:], in_=den[:])
                    # weighted messages, in place on g; self slot at [dd]
                    nc.vector.tensor_tensor(
                        out=g[:, 0:dd, :].rearrange("p d (h w) -> p d h w",
                                                    h=H1),
                        in0=g[:, 0:dd, :].rearrange("p d (h w) -> p d h w",
                                                    h=H1),
                        in1=ew[:, 0:dd, :, None].to_broadcast((P, dd, H1, HID)),
                        op=OP.mult)
                    nc.vector.tensor_tensor(
                        out=g[:, dd, :].rearrange("p (h w) -> p h w", h=H1),
                        in0=feat_own[:, r, :].rearrange("p (h w) -> p h w",
                                                        h=H1),
                        in1=ew[:, dd, :, None].to_broadcast((P, H1, HID)),
                        op=OP.mult)
                    x = epp.tile([P, C1], F32, tag="x")
                    nc.vector.reduce_sum(
                        out=x[:], in_=g[:].rearrange("p d c -> p c d"),
                        axis=mybir.AxisListType.X)
                    nc.vector.tensor_tensor(
                        out=x[:].rearrange("p (h w) -> p h w", h=H1),
                        in0=x[:].rearrange("p (h w) -> p h w", h=H1),
                        in1=den[:, :, None].to_broadcast((P, H1, HID)),
                        op=OP.mult)
                    nc.vector.tensor_tensor(out=x[:], in0=x[:], in1=b1r[:],
                                            op=OP.add)
                    # h1 = elu(x)
                    mn = epp.tile([P, C1], F32, tag="mn")
                    nc.vector.tensor_scalar_min(mn[:], x[:], 0.0)
                    nc.scalar.activation(out=mn[:], in_=mn[:], func=AF.Exp)
                    nc.vector.tensor_scalar_max(x[:], x[:], 0.0)
                    nc.vector.tensor_tensor(out=x[:], in0=x[:], in1=mn[:],
                                            op=OP.add)
                    h1r = epp.tile([P, C1], F16, tag="h1r")
                    nc.vector.tensor_scalar_sub(h1r[:], x[:], 1.0)
                    # fused layer-2 matmul for this round
                    tps = tpsp.tile([P, 2, P], F16)
                    for c in range(2):
                        nc.tensor.transpose(out=tps[:, c, :],
                                            in_=h1r[:, c * P:(c + 1) * P],
                                            identity=ident16[:])
                    h1t = h1tp.tile([P, 2, P], F16, tag="h1t")
                    nc.vector.tensor_copy(h1t[:], tps[:])
                    z2ps = z2psp.tile([P, OUT + 2], F32)
                    for c in range(2):
                        nc.tensor.matmul(z2ps[:], lhsT=h1t[:, c, :],
                                         rhs=w2e[:, c, :],
                                         start=(c == 0), stop=(c == 1))
                    nc.vector.tensor_copy(feat2_own[:, r, :], z2ps[:, 0:OUT])
                    nc.vector.tensor_copy(eler2_own[:, r, :],
                                          z2ps[:, OUT:OUT + 2])
                    nc.sync.dma_start(z2shard[r * 64:(r + 1) * 64, 0:OUT],
                                      feat2_own[0:64, r, :])
                    nc.sync.dma_start(
                        z2shard[HALFNT + r * 64:HALFNT + (r + 1) * 64, 0:OUT],
                        feat2_own[64:P, r, :])

            with nc.named_scope("ag2"):
                nc.gpsimd.collective_compute(
                    "AllGather", OP.bypass, replica_groups=[list(range(NCC))],
                    ins=[z2shard[0:HALFNT, :]], outs=[Z2[0:HALFT, :]])
                nc.gpsimd.collective_compute(
                    "AllGather", OP.bypass, replica_groups=[list(range(NCC))],
                    ins=[z2shard[HALFNT:NT, :]], outs=[Z2[HALFT:2 * HALFT, :]])

            # ---- layer-2 edge phase ----
            with (
                nc.named_scope("l2edge"),
                tc.tile_pool(name="fg2", bufs=4) as fg2p,
                tc.tile_pool(name="tmp2", bufs=2) as tmp2p,
                tc.tile_pool(name="ew2", bufs=6) as ew2p,
            ):
                for r in range(ROUNDS):
                    dd, da, db = DD[r], DA[r], DB[r]
                    o = int(moff[r])
                    c0 = int(gi_base[r]) // 16
                    g2 = fg2p.tile([P, dd + 1, Z2W], F16, tag="g2")
                    nc.gpsimd.dma_gather(
                        g2[:, 0:da, :], Z2[0:HALFT, :],
                        gidx[:, c0:c0 + da * 8], da * P, da * P, Z2W,
                        single_packet=False)
                    nc.gpsimd.dma_gather(
                        g2[:, da:dd, :], Z2[HALFT:2 * HALFT, :],
                        gidx[:, c0 + da * 8:c0 + dd * 8], db * P, db * P, Z2W,
                        single_packet=False)
                    tmp2 = tmp2p.tile([P, dd, OUT], F16, tag="tmp2")
                    nc.vector.tensor_tensor(
                        out=tmp2[:], in0=g2[:, 0:dd, 0:OUT],
                        in1=al2rep[:, None, :].to_broadcast((P, dd, OUT)),
                        op=OP.mult)
                    ew = ew2p.tile([P, dd + 1], F32, tag="ew2")
                    nc.vector.reduce_sum(out=ew[:, 0:dd], in_=tmp2[:],
                                         axis=mybir.AxisListType.X)
                    nc.vector.tensor_copy(ew[:, dd:dd + 1],
                                          eler2_own[:, r, 0:1])
                    if USE_LRELU:
                        nc.scalar.activation(out=ew[:], in_=ew[:],
                                             func=AF.Prelu,
                                             bias=eler2_own[:, r, 1:2],
                                             alpha=NEG_SLOPE)
                    else:
                        nc.vector.tensor_tensor(
                            out=ew[:], in0=ew[:],
                            in1=eler2_own[:, r, 1:2].to_broadcast((P, dd + 1)),
                            op=OP.add)
                        lr2 = ew2p.tile([P, dd + 1], F32, tag="lr2")
                        nc.vector.tensor_scalar_mul(lr2[:], ew[:], NEG_SLOPE)
                        nc.vector.tensor_tensor(out=ew[:], in0=ew[:],
                                                in1=lr2[:], op=OP.max)
                    nc.vector.tensor_tensor(
                        out=ew[:], in0=ew[:], in1=maskt[:, o:o + dd + 1],
                        op=OP.add)
                    den = ew2p.tile([P, 1], F32, tag="den2")
                    if USE_ACCUM:
                        nc.scalar.activation(out=ew[:], in_=ew[:], func=AF.Exp,
                                             accum_out=den[:])
                    else:
                        nc.scalar.activation(out=ew[:], in_=ew[:], func=AF.Exp)
                        nc.vector.reduce_sum(out=den[:], in_=ew[:],
                                             axis=mybir.AxisListType.X)
                    nc.vector.reciprocal(out=den[:], in_=den[:])
                    nc.vector.tensor_tensor(
                        out=g2[:, 0:dd, 0:OUT], in0=g2[:, 0:dd, 0:OUT],
                        in1=ew[:, 0:dd, None].to_broadcast((P, dd, OUT)),
                        op=OP.mult)
                    nc.vector.tensor_tensor(
                        out=g2[:, dd, 0:OUT], in0=feat2_own[:, r, :],
                        in1=ew[:, dd:dd + 1].to_broadcast((P, OUT)),
                        op=OP.mult)
                    ot = ew2p.tile([P, OUT], F32, tag="ot")
                    nc.vector.reduce_sum(
                        out=ot[:],
                        in_=g2[:, :, 0:OUT].rearrange("p d c -> p c d"),
                        axis=mybir.AxisListType.X)
                    nc.vector.tensor_tensor(
                        out=ot[:], in0=ot[:],
                        in1=den[:].to_broadcast((P, OUT)), op=OP.mult)
                    o16 = ew2p.tile([P, OUT], F16, tag="o16")
                    nc.vector.tensor_tensor(out=o16[:], in0=ot[:], in1=b2r[:],
                                            op=OP.add)
                    nc.sync.dma_start(outs["out"][:, r, :], o16[:])

    return kern


# -------------------------------------------------------------------- exec
def _run(sched, state):
    nc = bacc.Bacc("TRN2", target_bir_lowering=False, debug=False,
                   num_devices=NCC)
    ins_ap = {}
    for k, shape, dt in [
        ("W1ext", [IN_DIM, C1 + 2 * H1], F16),
        ("W2ext", [C1, OUT + 2], F16),
        ("alrep", [P, C1], F16),
        ("al2rep", [P, OUT], F16),
        ("ident16", [P, P], F16),
        ("b1rep", [P, C1], F32),
        ("b2rep", [P, OUT], F32),
        ("gidx", [16, sched["gi_len"] // 16], I16),
        ("maskt", [P, sched["SD2"]], F16),
        ("htiles", [P, ROUNDS * 2, P], F16),
    ]:
        ins_ap[k] = nc.dram_tensor(f"in_{k}", shape, dt,
                                   kind="ExternalInput").ap()
    outs_ap = {"out": nc.dram_tensor("out", [P, ROUNDS, OUT], F16,
                                     kind="ExternalOutput").ap()}
    kern = build_kernel_fn(sched)
    with tile.TileContext(nc) as tc:
        kern(tc, outs_ap, ins_ap)
    nc.compile()

    import jax
    from jax.sharding import Mesh, PartitionSpec
    from jax.experimental.shard_map import shard_map

    state["jax_thread"].join()
    if "jax_err" in state:
        raise state["jax_err"]

    partition_name = (nc.partition_id_tensor.name
                      if nc.partition_id_tensor else None)
    in_names, out_names, out_avals, zero_outs = [], [], [], []
    for alloc in nc.m.functions[0].allocations:
        if not isinstance(alloc, mybir.MemoryLocationSet):
            continue
        name = alloc.memorylocations[0].name
        if alloc.kind == "ExternalInput":
            if name != partition_name:
                in_names.append(name)
        elif alloc.kind == "ExternalOutput":
            out_names.append(name)
            shape = tuple(alloc.tensor_shape)
            dtype = mybir.dt.np(alloc.dtype)
            out_avals.append(jax.core.ShapedArray(shape, dtype))
            zero_outs.append(np.zeros(shape, dtype))
    n_params = len(in_names)
    n_outs = len(out_avals)
    in_names.extend(out_names)
    if partition_name is not None:
        in_names.append(partition_name)
    donate = tuple(range(n_params, n_params + n_outs))

    def _body(*args):
        operands = list(args)
        if partition_name is not None:
            operands.append(bass2jax.partition_id_tensor())
        outs = bass2jax._bass_exec_p.bind(
            *operands, out_avals=tuple(out_avals), in_names=tuple(in_names),
            out_names=tuple(out_names), lowering_input_output_aliases=(),
            sim_require_finite=True, sim_require_nnan=True, nc=nc)
        return tuple(outs)

    devices = state["devices"][:NCC]
    mesh = Mesh(np.asarray(devices), ("core",))
    in_specs = (PartitionSpec("core"),) * (n_params + n_outs)
    out_specs = (PartitionSpec("core"),) * len(out_names)
    sharded = jax.jit(
        shard_map(_body, mesh=mesh, in_specs=in_specs, out_specs=out_specs,
                  check_rep=False),
        donate_argnums=donate, keep_unused=True)

    state["marshal_thread"].join()
    if "marshal_err" in state:
        raise state["marshal_err"]
    concat = state["concat"]
    concat_in = [concat[name] for name in in_names[:n_params]]
    concat_zeros = [np.zeros((NCC * z.shape[0], *z.shape[1:]), z.dtype)
                    for z in zero_outs]
    out_arrs = sharded(*concat_in, *concat_zeros)
    res = np.asarray(out_arrs[0])          # single device->host transfer
    return res


def assemble_output(res, sched):
    # res: [NCC*P, ROUNDS, OUT] f16, core-major on axis 0
    big = (res.reshape(NCC, P, ROUNDS, OUT).transpose(0, 2, 1, 3)
           .reshape(NCC * NT, OUT).astype(np.float32))
    cn2 = sched["core_nodes"].reshape(-1)
    vv = cn2 >= 0
    out = np.zeros((N, OUT), np.float32)
    out[cn2[vv]] = big[vv]
    return out


def kernel(**inputs) -> np.ndarray:
    state = {}
    jt = threading.Thread(target=_init_jax, args=(state,))
    jt.start()
    state["jax_thread"] = jt
    src = np.asarray(inputs["src"]).astype(np.int64)
    dst = np.asarray(inputs["dst"]).astype(np.int64)
    sched = schedule(src, dst)
    mt = threading.Thread(target=marshal, args=(inputs, sched, state))
    mt.start()
    state["marshal_thread"] = mt
    res = _run(sched, state)
    return assemble_output(res, sched)


if __name__ == "__main__":
    import pickle, time
    with open("/tmp/inputs.pkl", "rb") as f:
        inputs = pickle.load(f)
    t0 = time.perf_counter()
    out = kernel(**inputs)
    t1 = time.perf_counter()
    print(f"kernel wall {t1-t0:.2f}s")
    exp = np.load("/tmp/expected_np.npy")
    rel = np.linalg.norm(out - exp) / np.linalg.norm(exp)
    print("Relative error:", rel)
